# revision 21
# baseline (speedup 1.0000x reference)
"""Trainium2 Bass kernel for PixelUnshuffle->MHA->PixelShuffle (nn_Attention).

Reference computation (per batch element, 8 batch elements data-parallel
across 8 NeuronCores):
  x [64, 256, 256] --PixelUnshuffle(8)--> tokens [N=1024, C=4096]
  qkv = tokens @ W_qkv            [1024, 768]
  4-head attention (d=64), softmax over tokens
  y = attn_out @ W_out + b_out    [1024, 4096]
  --PixelShuffle(8)--> [64, 256, 256]

Key layout trick: the pixel un/shuffle is folded into the matmuls.
x is DMA'd in its natural (fully contiguous) layout, then a single
cast+de-stride engine copy produces bf16 tiles in (r2, hh, ww) order so
every matmul operand is contiguous. The QKV projection iterates over the
inner shuffle coordinate r2 (8 values); the output projection writes its
PSUM results back through strided evacuation copies straight into the
pixel-shuffled output layout. All DMA transfers move >=1KB contiguous
runs. Weights are host-side pre-permuted and pre-cast to bf16.

Token index   n = hh*32 + ww            (hh, ww in [0,32))
Channel index c = c0*64 + r1*8 + r2     (c0 in [0,64), r1, r2 in [0,8))
x[c0, hh*8+r1, ww*8+r2] = tokens[n, c]

Attention is computed transposed (dotsT[m, n] with the summed token m on
partitions) so that no on-chip transposes are needed anywhere:
  dotsT[m,n] = kT_h[:, m-chunk] (lhsT) x qT_h (rhs)  -> exp -> edotsT
  out_augT[i', n] = sum_m v_aug[m, i'] * edotsT[m, n]  with v_aug
  carrying an extra ones column so row 64 accumulates the softmax
  denominator Z[n] for free. 1/Z is computed on 64 lanes by bouncing Z
  through DRAM ([1,1024] -> [64,16]), and broadcast across partitions
  with a 0-stride DRAM load.
"""

import sys

if "/opt/trn_rl_repo" not in sys.path:
    sys.path.insert(0, "/opt/trn_rl_repo")

import os

import ml_dtypes
import numpy as np

import concourse.bass as bass
from concourse import bacc, mybir, tile
from concourse.bass_utils import run_bass_kernel_spmd

F32 = mybir.dt.float32
BF16 = mybir.dt.bfloat16

SCALE = 0.125  # DIM_HEAD ** -0.5

_CACHE = {}


def _build(debug_outs=False):
    nc = bacc.Bacc("TRN2", target_bir_lowering=False, debug=False, num_devices=8)

    x_d = nc.dram_tensor("x", [64, 256, 256], F32, kind="ExternalInput").ap()
    wq_d = nc.dram_tensor("W_qkv", [4096, 768], BF16, kind="ExternalInput").ap()
    wo_d = nc.dram_tensor("W_out", [256, 4096], BF16, kind="ExternalInput").ap()
    b_d = nc.dram_tensor("b_out", [4096], F32, kind="ExternalInput").ap()
    out_d = nc.dram_tensor("out", [64, 256, 256], F32, kind="ExternalOutput").ap()

    zsc_d = nc.dram_tensor("z_scratch", [4, 1024], F32).ap()
    zrc_d = nc.dram_tensor("zr_scratch", [4, 1024], F32).ap()

    dbg = None
    if debug_outs:
        dbg = {
            "qkT": nc.dram_tensor(
                "dbg_qkT", [128, 4, 1024], F32, kind="ExternalOutput"
            ).ap(),
            "v_sb": nc.dram_tensor(
                "dbg_v", [128, 8, 4, 68], F32, kind="ExternalOutput"
            ).ap(),
            "outT": nc.dram_tensor(
                "dbg_outT", [128, 2, 1024], F32, kind="ExternalOutput"
            ).ap(),
            "zbc": nc.dram_tensor(
                "dbg_zbc", [64, 4, 1024], F32, kind="ExternalOutput"
            ).ap(),
        }

    def dram_ap(base, off, pattern):
        return bass.AP(tensor=base.tensor, offset=base.offset + off, ap=pattern)

    with tile.TileContext(nc) as tc:
        _build_tiled(nc, tc, x_d, wq_d, wo_d, b_d, out_d, zsc_d, zrc_d, dram_ap, dbg)
    nc.compile()
    return nc


def _build_tiled(nc, tc, x_d, wq_d, wo_d, b_d, out_d, zsc_d, zrc_d, dram_ap, dbg=None):
    from contextlib import ExitStack

    with ExitStack() as ctx:
        pers = ctx.enter_context(tc.tile_pool(name="pers", bufs=1))
        s23 = ctx.enter_context(tc.tile_pool(name="s23", bufs=1))
        s2 = ctx.enter_context(tc.tile_pool(name="s2", bufs=1))
        psA_stack = ExitStack()
        psA = psA_stack.enter_context(tc.tile_pool(name="psA", bufs=1, space="PSUM"))

        # ---- persistent tiles ----
        # qkT[d-part, ot, n] : ot 0,1 = q dims 0..128,128..256; ot 2,3 = k
        qkT = pers.tile([128, 4, 1024], BF16)
        # v_aug[m-part, mc, h, 68] bf16, col 64 = ones (65-67 pad for align)
        v_sb = pers.tile([128, 8, 4, 68], BF16)
        # outT[i-part, ic, n] : i = h*64+d ; ic = i//128
        outT = pers.tile([128, 2, 1024], BF16)
        # bias[c-part, r2, cg]
        bias_sb = pers.tile([128, 8, 4], F32)

        nc.vector.memset(v_sb[:, :, :, 64:68], 1.0)
        # host pre-arranges b_out as [p, r2, cg] so this is a flat copy
        nc.sync.dma_start(
            out=bias_sb[:],
            in_=dram_ap(b_d, 0, [[32, 128], [4, 8], [1, 4]]),
        )

        # W_out preloads early (outer pool so it survives into stage 3)
        wo_sb = s23.tile([128, 2, 4096], BF16)  # [i-part, ic, c_perm]
        nc.sync.dma_start(
            out=wo_sb[:],
            in_=dram_ap(wo_d, 0, [[4096, 128], [524288, 2], [1, 4096]]),
        )

        # =========================== stage 1 ===========================
        # QKV projection with pixel-unshuffle folded in. 2 windows of 512
        # tokens (hh-halves).
        with (
            tc.tile_pool(name="wq", bufs=1) as wqp,
            tc.tile_pool(name="xw", bufs=1) as xwp,
        ):
            wq_sb = wqp.tile([128, 8, 4, 768], BF16)  # [c-part, r2, cg, o]
            for r2 in range(8):
                nc.sync.dma_start(
                    out=wq_sb[:, r2, :, :],
                    in_=dram_ap(
                        wq_d, r2 * 512 * 768, [[768, 128], [98304, 4], [1, 768]]
                    ),
                )

            for w in range(2):
                xtbs = []
                for cg in range(4):
                    xt = xwp.tile([128, 16, 32, 8], F32, tag="xt", bufs=2)
                    # one DMA per hh row-block: DRAM run (r1, w) = 2048 contig
                    for hh in range(16):
                        eng = (nc.sync, nc.scalar, nc.sync, nc.gpsimd)[hh % 4]
                        eng.dma_start(
                            out=xt[:, hh, :, :],
                            in_=dram_ap(
                                x_d,
                                cg * 16 * 65536 + (w * 16 + hh) * 2048,
                                [[65536, 16], [1, 2048]],
                            ),
                        )
                    # cast f32 -> bf16 AND de-stride: (hh, ww, r2) -> (r2, hh, ww)
                    xtb = xwp.tile([128, 8, 16, 32], BF16, tag="xtb", bufs=5)
                    src = xt[:].transpose([0, 3, 1, 2])
                    if cg % 2 == 0:
                        nc.vector.tensor_copy(xtb[:], src)
                    else:
                        nc.scalar.copy(xtb[:], src)
                    xtbs.append(xtb)

                # q/k tiles: out[o-slice, tokens]
                for ot in range(4):
                    qk_ps = psA.tile([128, 512], F32, tag="qk", bufs=1)
                    for cg in range(4):
                        for r2 in range(8):
                            nc.tensor.matmul(
                                qk_ps[:],
                                wq_sb[:, r2, cg, ot * 128 : (ot + 1) * 128],
                                xtbs[cg][:, r2, :, :],
                                start=(cg == 0 and r2 == 0),
                                stop=(cg == 3 and r2 == 7),
                            )
                    dst = qkT[:, ot, w * 512 : (w + 1) * 512]
                    if ot % 2 == 0:
                        nc.scalar.copy(dst, qk_ps[:])
                    else:
                        nc.vector.tensor_copy(dst, qk_ps[:])

                # v tiles: out[tokens, i]
                for s in range(4):
                    v_ps = psA.tile([128, 256], F32, tag="v", bufs=1)
                    for cg in range(4):
                        for r2 in range(8):
                            nc.tensor.matmul(
                                v_ps[:],
                                xtbs[cg][:, r2, 4 * s : 4 * s + 4, :],
                                wq_sb[:, r2, cg, 512:768],
                                start=(cg == 0 and r2 == 0),
                                stop=(cg == 3 and r2 == 7),
                            )
                    mtg = 4 * w + s
                    nc.vector.tensor_copy(
                        v_sb[:, mtg, :, 0:64],
                        v_ps[:].rearrange("p (h d) -> p h d", h=4),
                    )

        if dbg is not None:
            nc.gpsimd.dma_start(out=dbg["qkT"][:], in_=qkT[:])
            nc.gpsimd.dma_start(out=dbg["v_sb"][:], in_=v_sb[:])

        # =========================== stage 2: attention ===========================
        for hp in range(2):  # head pair: heads 2*hp, 2*hp+1
            ed = [
                s2.tile(
                    [128, 8, 1024], BF16, tag="edots", bufs=2, name=f"ed_{hp}_{i}"
                )
                for i in range(2)
            ]
            oaug = [
                psA.tile(
                    [128, 2, 512], F32, tag="oaug", bufs=2, name=f"oaug_{hp}_{i}"
                )
                for i in range(2)
            ]
            for mc in range(8):
                for nh in range(2):
                    for h2 in range(2):
                        h = 2 * hp + h2
                        b = h2 * 64
                        dt_ps = psA.tile([128, 512], F32, tag="dt", bufs=2)
                        nc.tensor.matmul(
                            dt_ps[:],
                            qkT[b : b + 64, 2 + hp, mc * 128 : (mc + 1) * 128],
                            qkT[b : b + 64, hp, nh * 512 : (nh + 1) * 512],
                            start=True,
                            stop=True,
                        )
                        nc.scalar.activation(
                            ed[h2][:, mc, nh * 512 : (nh + 1) * 512],
                            dt_ps[:],
                            mybir.ActivationFunctionType.Exp,
                            scale=SCALE,
                        )
                        nc.tensor.matmul(
                            oaug[h2][0:68, nh, :],
                            v_sb[:, mc, h, :],
                            ed[h2][:, mc, nh * 512 : (nh + 1) * 512],
                            start=(mc == 0),
                            stop=(mc == 7),
                        )
            # normalize: out[d, n] * (1/Z[n]) ; Z = row 64 of oaug
            for h2 in range(2):
                h = 2 * hp + h2
                # Z row (psum, 1 lane) -> sbuf -> DRAM
                zrow = s2.tile([65, 1024], F32, tag="zrow", bufs=2)
                nc.vector.tensor_copy(
                    zrow[64:65, :],
                    oaug[h2][64:65, :, :].rearrange("p a b -> p (a b)"),
                )
                nc.sync.dma_start(out=zsc_d[h, :], in_=zrow[64:65, :])
                # reload as [64, 16] so reciprocal runs on 64 lanes
                z16 = s2.tile([64, 16], F32, tag="z16", bufs=2)
                nc.sync.dma_start(
                    out=z16[:],
                    in_=dram_ap(zsc_d, h * 1024, [[16, 64], [1, 16]]),
                )
                z16r = s2.tile([64, 16], F32, tag="z16r", bufs=2)
                nc.vector.reciprocal(z16r[:], z16[:])
                nc.sync.dma_start(
                    out=zrc_d[h, :].rearrange("(a b) -> a b", a=64), in_=z16r[:]
                )
                # broadcast-load 1/Z to all 64 partitions
                zbc = s2.tile([64, 1024], F32, tag="zbc", bufs=2)
                nc.sync.dma_start(
                    out=zbc[:],
                    in_=dram_ap(zrc_d, h * 1024, [[0, 64], [1, 1024]]),
                )
                if dbg is not None:
                    nc.sync.dma_start(out=dbg["zbc"][:, h, :], in_=zbc[:])
                if h2 == 0:
                    nc.vector.tensor_mul(
                        outT[0:64, hp, :],
                        oaug[h2][0:64, :, :].rearrange("p a b -> p (a b)"),
                        zbc[:],
                    )
                else:
                    onrm = s2.tile([64, 1024], BF16, tag="onrm", bufs=2)
                    nc.vector.tensor_mul(
                        onrm[:],
                        oaug[h2][0:64, :, :].rearrange("p a b -> p (a b)"),
                        zbc[:],
                    )
                    nc.sync.dma_start(out=outT[64:128, hp, :], in_=onrm[:])

        if dbg is not None:
            nc.gpsimd.dma_start(out=dbg["outT"][:], in_=outT[:])

        psA_stack.close()

        # ---------------- stage 3: output projection ----------------
        with (
            tc.tile_pool(name="s3", bufs=1) as s3,
            tc.tile_pool(name="ps3", bufs=1, space="PSUM") as ps3,
        ):
            for ct in range(4):
                y_t = s3.tile([128, 32, 32, 8], F32, tag="yt", bufs=2)
                for nq in range(4):
                    # y_big holds all 8 r2 for this (ct, nq): 4 banks
                    y_big = ps3.tile([128, 8, 256], F32, tag="ybig", bufs=2)
                    for r2 in range(8):
                        for ic in range(2):
                            # one start/stop per PSUM bank (r2 pairs share)
                            nc.tensor.matmul(
                                y_big[:, r2, :],
                                wo_sb[
                                    :,
                                    ic,
                                    r2 * 512 + ct * 128 : r2 * 512 + (ct + 1) * 128,
                                ],
                                outT[:, ic, nq * 256 : (nq + 1) * 256],
                                start=(r2 % 2 == 0 and ic == 0),
                                stop=(r2 % 2 == 1 and ic == 1),
                            )
                    # single evacuation: bias add + (r2, hhq, ww)->(hhq, ww, r2)
                    dst = y_t[:, nq * 8 : (nq + 1) * 8, :, :]
                    src = (
                        y_big[:]
                        .rearrange("p r (a b) -> p r a b", a=8)
                        .transpose([0, 2, 3, 1])
                    )
                    bias_bc = bias_sb[:, None, None, :, ct].broadcast_to(
                        [128, 8, 32, 8]
                    )
                    nc.vector.tensor_add(dst, src, bias_bc)
                for hh in range(32):
                    eng = (nc.sync, nc.scalar, nc.sync, nc.gpsimd)[hh % 4]
                    eng.dma_start(
                        out=dram_ap(
                            out_d,
                            ct * 16 * 65536 + hh * 2048,
                            [[65536, 16], [1, 2048]],
                        ),
                        in_=y_t[:, hh, :, :],
                    )


def _get_nc():
    if "nc" not in _CACHE:
        _CACHE["nc"] = _build()
    return _CACHE["nc"]


def _prep_weights(W_qkv, W_out, b_out):
    wq_perm = np.ascontiguousarray(
        W_qkv.reshape(64, 8, 8, 768).transpose(2, 0, 1, 3).reshape(4096, 768)
    ).astype(ml_dtypes.bfloat16)
    wo_perm = np.ascontiguousarray(
        W_out.reshape(256, 64, 8, 8).transpose(0, 3, 1, 2).reshape(256, 4096)
    ).astype(ml_dtypes.bfloat16)
    # b_perm[r2*512 + c0*8 + r1] = b_out[c0*64 + r1*8 + r2], then laid out
    # [p, r2, cg] where p = (c0 % 16)*8 + r1, cg = c0 // 16
    b_perm = b_out.reshape(64, 8, 8).transpose(2, 0, 1).reshape(4096)
    b_perm = np.ascontiguousarray(
        b_perm.reshape(8, 4, 128).transpose(2, 0, 1).reshape(4096)
    ).astype(np.float32)
    return wq_perm, wo_perm, b_perm


def kernel(x, W_qkv, W_out, b_out):
    nc = _get_nc()
    wq_perm, wo_perm, b_perm = _prep_weights(
        np.asarray(W_qkv, dtype=np.float32),
        np.asarray(W_out, dtype=np.float32),
        np.asarray(b_out, dtype=np.float32),
    )

    in_maps = [
        {
            "x": np.ascontiguousarray(x[b]).astype(np.float32),
            "W_qkv": wq_perm,
            "W_out": wo_perm,
            "b_out": b_perm,
        }
        for b in range(8)
    ]
    trace = bool(int(os.environ.get("BENCH_TRACE", "0")))
    res = run_bass_kernel_spmd(nc, in_maps, core_ids=list(range(8)), trace=trace)
    if trace:
        _CACHE["last_result"] = res
    return np.stack([res.results[b]["out"] for b in range(8)]).astype(np.float32)
